# revision 11
# baseline (speedup 1.0000x reference)
"""KuraNet Trainium2 kernel.

Pipeline (8 NeuronCores, SPMD, core c owns pair-rows i in [128c, 128c+128)):
  - L1 of the pair-MLP is separable: h1[(i,j),f] = x_i.W1A_f + x_j.W1B_f, so it
    reduces to two tiny [1024,16]@[16,128] matmuls (u, v).
  - BN1 batch stats over the N^2 Cartesian pair grid are exact in closed form
    from the 16x16 covariance of x (cross-covariance over the product grid
    vanishes), so no pass over N^2 is needed for BN1.
  - Pass 1 over the core's 131072 pairs (all matmuls bf16): g = Lrelu(av+su_i)
    on ACT (bf16, with accum giving Sum(g) so Sum(h2) = W2^T Sum(g) is free);
    h2 = W2^T g on PE; sumsq(h2) via DVE PSUM->SBUF copy + Pool squaring stt
    (Pool cannot access PSUM).
  - One AllGather of per-core (sum, sumsq) -> exact global BN2 stats.
  - Pass 2: rebuild g, h2 = W2^T g; BN2 affine y = a2*h2+c2 on DVE (PSUM
    read), Lrelu on Pool -> g2 bf16; k = g2^T w3 with g2 blocks stationary so
    k lands partition-major (transposed block layout).
  - One AllGather of k blocks; softmax WITHOUT the max-subtraction pass
    (logits are BN-bounded; exp is computed directly in fp32 and is
    mathematically identical after normalization). The softmax scale
    0.5*N/Z is NOT applied to K at all -- it is folded into the Euler-step
    scalar (alpha*0.5/Z), so the resident coupling matrix is just
    KSb = exp(k) + exp(k)^T in bf16 (built with 64 PE transposes + DVE adds).
  - 150 explicit-Euler Kuramoto steps, replicated per core (no per-step
    collectives): theta' = theta + (0.5a/Z)*(cos th*(KSb@sin th) -
    sin th*(KSb@cos th)); KSb@[s|c] as 64 accumulating [128,128]x[128,2]
    matmuls with KSb blocks stationary (KSb symmetric-transposed blocks give
    the needed transpose for free). sin/cos via one ACT Sin directly into
    bf16 after a DVE range-wrap; combine + Euler update on DVE.
Output traj is produced partition-major [128, 150, 8] and unsharded on host.
"""

import math

import numpy as np

import concourse.bass as bass
import concourse.bacc as bacc
import concourse.tile as tile
import concourse.mybir as mybir
from concourse.bass_utils import run_bass_kernel_spmd

N = 1024
FD = 16
H = 128
P = 128
NB = 8
NCORES = 8
STEPS = 150
ALPHA = 0.1
EPS = 1e-5
SLOPE = 0.01
PI = math.pi
F32 = mybir.dt.float32
BF16 = mybir.dt.bfloat16
AF = mybir.ActivationFunctionType
OP = mybir.AluOpType
AX = mybir.AxisListType


def build_program(steps=STEPS, debug=False):
    nc = bacc.Bacc("TRN2", target_bir_lowering=False, debug=False,
                   num_devices=NCORES)
    ins = {}
    for name, sh in [("xT", [FD, N]), ("x8", [NB, P, FD]), ("xbT", [FD, P]),
                     ("w1a", [FD, H]), ("w1b", [FD, H]), ("w2", [H, H]),
                     ("w3l", [H, 1]), ("b1", [H, 1]), ("g1", [H, 1]),
                     ("be1", [H, 1]), ("g2", [H, 1]), ("be2", [H, 1]),
                     ("ident", [P, P]), ("theta0", [P, NB])]:
        ins[name] = nc.dram_tensor(name, sh, F32, kind="ExternalInput")
    traj_ext = nc.dram_tensor("traj_pb", [P, steps, NB], F32,
                              kind="ExternalOutput")
    ksym_ext = (nc.dram_tensor("ksym_dbg", [P, NB, N], F32,
                               kind="ExternalOutput") if debug else None)

    stats_in = nc.dram_tensor("stats_in", [H, 2], F32)
    stats_sh = nc.dram_tensor("stats_sh", [NCORES, H, 2], F32,
                              addr_space="Shared")
    k_in = nc.dram_tensor("k_in", [P, N], F32)
    k_sh = nc.dram_tensor("k_sh", [NCORES, P, N], F32, addr_space="Shared")
    rg = [list(range(NCORES))]

    from contextlib import ExitStack
    with tile.TileContext(nc) as tc, ExitStack() as ctx:
        const = ctx.enter_context(tc.tile_pool(name="const", bufs=1))
        big = ctx.enter_context(tc.tile_pool(name="big", bufs=1))
        work = ctx.enter_context(tc.tile_pool(name="work", bufs=3))
        g2p = ctx.enter_context(tc.tile_pool(name="g2p", bufs=2))
        small = ctx.enter_context(tc.tile_pool(name="small", bufs=1))
        ps = ctx.enter_context(tc.tile_pool(name="ps", bufs=2, space="PSUM"))
        pss = ctx.enter_context(tc.tile_pool(name="pss", bufs=1, space="PSUM"))
        psk = ctx.enter_context(tc.tile_pool(name="psk", bufs=1, space="PSUM"))
        pso = ctx.enter_context(tc.tile_pool(name="pso", bufs=1, space="PSUM"))

        def load(name, sh):
            t = const.tile(sh, F32, tag=name)
            nc.sync.dma_start(out=t[:], in_=ins[name][:])
            return t

        sXT = load("xT", [FD, N])
        sXBT = load("xbT", [FD, P])
        sW1A = load("w1a", [FD, H])
        sW1B = load("w1b", [FD, H])
        sW2 = load("w2", [H, H])
        sW3 = load("w3l", [H, 1])
        sB1 = load("b1", [H, 1])
        sG1 = load("g1", [H, 1])
        sBE1 = load("be1", [H, 1])
        sG2 = load("g2", [H, 1])
        sBE2 = load("be2", [H, 1])
        sID = load("ident", [P, P])
        sX8 = const.tile([P, NB, FD], F32, tag="x8")
        nc.sync.dma_start(out=sX8[:], in_=ins["x8"][:].rearrange("b p k -> p b k"))

        # bf16 copies of TensorEngine-facing constants
        sW2b = const.tile([H, H], BF16, tag="w2b")
        nc.vector.tensor_copy(sW2b[:], sW2[:])
        sW3b = const.tile([H, 1], BF16, tag="w3b")
        nc.vector.tensor_copy(sW3b[:], sW3[:])
        sIDb = const.tile([P, P], BF16, tag="identb")
        nc.vector.tensor_copy(sIDb[:], sID[:])

        # ---- BN1 closed-form setup ----
        xb = small.tile([FD, 1], F32)
        nc.vector.tensor_reduce(out=xb[:], in_=sXT[:], axis=AX.X, op=OP.add)
        nc.vector.tensor_scalar_mul(xb[:], xb[:], 1.0 / N)

        mA = small.tile([H, 1], F32)
        mB = small.tile([H, 1], F32)
        for w, m in ((sW1A, mA), (sW1B, mB)):
            pm = pss.tile([H, 1], F32, tag="setup")
            nc.tensor.matmul(pm[:], w[:], xb[:], start=True, stop=True)
            nc.vector.tensor_copy(m[:], pm[:])
        m1 = small.tile([H, 1], F32)   # mu1 + b1
        nc.vector.tensor_add(m1[:], mA[:], mB[:])
        nc.vector.tensor_add(m1[:], m1[:], sB1[:])

        pS = pss.tile([FD, FD], F32, tag="setup")
        for b in range(NB):
            nc.tensor.matmul(pS[:], sX8[:, b, :], sX8[:, b, :],
                             start=(b == 0), stop=(b == NB - 1))
        sS = small.tile([FD, FD], F32)
        nc.vector.tensor_copy(sS[:], pS[:])
        ones = small.tile([P, 1], F32)
        nc.vector.memset(ones[:], 1.0)
        pqs = pss.tile([1, H], F32, tag="setupB")
        for half, w in enumerate((sW1A, sW1B)):
            pSA = pss.tile([FD, H], F32, tag="setup")
            nc.tensor.matmul(pSA[:], sS[:], w[:], start=True, stop=True)
            qa = small.tile([FD, H], F32, tag=f"qa{half}")
            nc.vector.tensor_mul(qa[:], pSA[:], w[:])
            nc.tensor.matmul(pqs[:], ones[0:FD, :], qa[:],
                             start=(half == 0), stop=(half == 1))
        qsum = small.tile([1, H], F32)
        nc.vector.tensor_copy(qsum[:], pqs[:])
        pq = pss.tile([H, 1], F32, tag="setup")
        nc.tensor.transpose(pq[:], qsum[:], sID[0:1, 0:1])
        t1 = small.tile([H, 1], F32, tag="t1")
        nc.vector.tensor_mul(t1[:], mA[:], mA[:])
        var1 = small.tile([H, 1], F32)
        nc.vector.scalar_tensor_tensor(out=var1[:], in0=pq[:], scalar=1.0 / N,
                                       in1=t1[:], op0=OP.mult, op1=OP.subtract)
        nc.vector.tensor_mul(t1[:], mB[:], mB[:])
        nc.vector.tensor_sub(var1[:], var1[:], t1[:])
        eps_t = small.tile([H, 1], F32)
        nc.vector.memset(eps_t[:], EPS)
        sd = small.tile([H, 1], F32)
        nc.scalar.activation(out=sd[:], in_=var1[:], func=AF.Sqrt, bias=eps_t[:])
        a1 = small.tile([H, 1], F32)
        nc.vector.reciprocal(a1[:], sd[:])
        nc.vector.tensor_mul(a1[:], a1[:], sG1[:])
        c1 = small.tile([H, 1], F32)
        nc.vector.tensor_mul(c1[:], a1[:], m1[:])
        nc.vector.tensor_sub(c1[:], sBE1[:], c1[:])

        pu = pss.tile([H, P], F32, tag="setup")
        nc.tensor.matmul(pu[:], sW1A[:], sXBT[:], start=True, stop=True)
        su = const.tile([H, P], F32, tag="su")
        nc.scalar.activation(out=su[:], in_=pu[:], func=AF.Identity,
                             bias=c1[:], scale=a1[:])
        av = const.tile([H, N], F32, tag="av")
        for h in range(2):
            pv = pss.tile([H, 512], F32, tag="setup")
            nc.tensor.matmul(pv[:], sW1B[:], sXT[:, h * 512:(h + 1) * 512],
                             start=True, stop=True)
            nc.scalar.activation(out=av[:, h * 512:(h + 1) * 512], in_=pv[:],
                                 func=AF.Identity, scale=a1[:])

        # ---- pass 1: BN2 moments via DVE bn_stats (the only engine that can
        # both read PSUM and form mean/var in one pass) ----
        stats = big.tile([P, P, 2, 6], F32, tag="stats")
        for i in range(P):
            g = work.tile([H, N], BF16, tag="g")
            nc.scalar.activation(out=g[:], in_=av[:], func=AF.Lrelu,
                                 bias=su[:, i:i + 1], alpha=SLOPE)
            for h in range(2):
                ph2 = ps.tile([H, 512], F32, tag="ph2")
                nc.tensor.matmul(ph2[:], sW2b[:], g[:, h * 512:(h + 1) * 512],
                                 start=True, stop=True)
                nc.vector.bn_stats(out=stats[:, i, h, :], in_=ph2[:])
        mv = small.tile([H, 2], F32)
        nc.vector.bn_aggr(out=mv[:], in_=stats[:].rearrange("p i h s -> p (i h) s"))
        CNT = float(P * N)
        ex = small.tile([H, 2], F32)
        nc.vector.tensor_scalar_mul(ex[:, 0:1], mv[:, 0:1], CNT)
        tq = small.tile([H, 1], F32, tag="tq")
        nc.vector.tensor_mul(tq[:], mv[:, 0:1], mv[:, 0:1])
        nc.vector.tensor_add(tq[:], tq[:], mv[:, 1:2])
        nc.vector.tensor_scalar_mul(ex[:, 1:2], tq[:], CNT)
        nc.sync.dma_start(out=stats_in[:], in_=ex[:])
        nc.gpsimd.collective_compute("AllGather", OP.bypass, replica_groups=rg,
                                     ins=[stats_in[:]], outs=[stats_sh[:]])
        sg = small.tile([H, NCORES, 2], F32)
        nc.sync.dma_start(out=sg[:], in_=stats_sh[:].rearrange("r p s -> p r s"))
        tot = small.tile([H, 2], F32)
        nc.vector.tensor_reduce(out=tot[:, 0:1], in_=sg[:, :, 0], axis=AX.X,
                                op=OP.add)
        nc.vector.tensor_reduce(out=tot[:, 1:2], in_=sg[:, :, 1], axis=AX.X,
                                op=OP.add)
        TOT = float(NCORES * P * N)
        mean2 = small.tile([H, 1], F32)
        nc.vector.tensor_scalar_mul(mean2[:], tot[:, 0:1], 1.0 / TOT)
        var2 = small.tile([H, 1], F32)
        nc.vector.tensor_scalar_mul(var2[:], tot[:, 1:2], 1.0 / TOT)
        tm = small.tile([H, 1], F32, tag="tm")
        nc.vector.tensor_mul(tm[:], mean2[:], mean2[:])
        nc.vector.tensor_sub(var2[:], var2[:], tm[:])
        sd2 = small.tile([H, 1], F32)
        nc.scalar.activation(out=sd2[:], in_=var2[:], func=AF.Sqrt, bias=eps_t[:])
        a2 = small.tile([H, 1], F32)
        nc.vector.reciprocal(a2[:], sd2[:])
        nc.vector.tensor_mul(a2[:], a2[:], sG2[:])
        c2 = small.tile([H, 1], F32)
        nc.vector.tensor_mul(c2[:], a2[:], mean2[:])
        nc.vector.tensor_sub(c2[:], sBE2[:], c2[:])

        # ---- pass 2: k logits (transposed block layout) ----
        pkb0 = psk.tile([P, NB, 64], F32, tag="pk0")
        pkb1 = psk.tile([P, NB, 64], F32, tag="pk1")
        pkb = [pkb0, pkb1]

        def build_g(i):
            g = work.tile([H, N], BF16, tag="g")
            nc.scalar.activation(out=g[:], in_=av[:], func=AF.Lrelu,
                                 bias=su[:, i:i + 1], alpha=SLOPE)
            return g

        # Software-pipelined: ACT builds g(i+1) before consuming ph2a(i), so
        # the ACT->PE->ACT chain of one iteration hides under the next g-build.
        g_cur = build_g(0)
        for i in range(P):
            ph2a = ps.tile([H, 512], F32, tag="ph2")
            nc.tensor.matmul(ph2a[:], sW2b[:], g_cur[:, 0:512],
                             start=True, stop=True)
            ph2b = ps.tile([H, 512], F32, tag="ph2")
            nc.tensor.matmul(ph2b[:], sW2b[:], g_cur[:, 512:1024],
                             start=True, stop=True)
            g_next = build_g(i + 1) if i + 1 < P else None
            g2t = g2p.tile([H, N], BF16, tag="g2")
            # ACT: fused BN2 affine + Lrelu on cols [0:448] (PSUM half A);
            # DVE 2-op path covers [448:512] of half A and all of half B.
            nc.scalar.activation(out=g2t[:, 0:448], in_=ph2a[:, 0:448],
                                 func=AF.Lrelu, bias=c2[:], scale=a2[:],
                                 alpha=SLOPE)
            yta = work.tile([H, 64], F32, tag="yta")
            nc.vector.tensor_scalar(out=yta[:], in0=ph2a[:, 448:512],
                                    scalar1=a2[:], scalar2=c2[:],
                                    op0=OP.mult, op1=OP.add)
            nc.vector.scalar_tensor_tensor(out=g2t[:, 448:512],
                                           in0=yta[:], scalar=SLOPE,
                                           in1=yta[:], op0=OP.mult, op1=OP.max)
            yt = work.tile([H, 512], F32, tag="yt")
            nc.vector.tensor_scalar(out=yt[:], in0=ph2b[:], scalar1=a2[:],
                                    scalar2=c2[:], op0=OP.mult, op1=OP.add)
            nc.vector.scalar_tensor_tensor(out=g2t[:, 512:1024],
                                           in0=yt[:], scalar=SLOPE, in1=yt[:],
                                           op0=OP.mult, op1=OP.max)
            bank, slot = divmod(i, 64)
            for jb in range(NB):
                nc.tensor.matmul(pkb[bank][:, jb, slot:slot + 1],
                                 g2t[:, jb * P:(jb + 1) * P], sW3b[:],
                                 start=True, stop=True)
            g_cur = g_next
        KT = big.tile([P, NB, P], F32, tag="KT")
        for bank in range(2):
            nc.vector.tensor_copy(KT[:, :, bank * 64:(bank + 1) * 64],
                                  pkb[bank][:])
        nc.sync.dma_start(out=k_in[:], in_=KT[:].rearrange("p j f -> p (j f)"))
        nc.gpsimd.collective_compute("AllGather", OP.bypass, replica_groups=rg,
                                     ins=[k_in[:]], outs=[k_sh[:]])
        # kallT[p, r, s, f] = k(128r+f, 128s+p)
        kallT = big.tile([P, NB, NB, P], F32, tag="kallT")
        nc.sync.dma_start(out=kallT[:],
                          in_=k_sh[:].rearrange("r p (s f) -> p r s f", s=NB))

        # ---- exp (no max pass; normalization folded into the step scalar),
        #      symmetrize: KSb_blk(a,b) = exp(k)_blk(b,a) + T(exp(k)_blk(a,b))
        ET = big.tile([P, NB, NB, P], BF16, tag="ET")
        es = small.tile([P, NB], F32)
        for r in range(NB):
            nc.scalar.activation(out=ET[:, r, :, :], in_=kallT[:, r, :, :],
                                 func=AF.Exp, accum_out=es[:, r:r + 1])
        rs = small.tile([P, 1], F32)
        nc.vector.tensor_reduce(out=rs[:], in_=es[:], axis=AX.X, op=OP.add)
        pz = pss.tile([1, 1], F32, tag="setup")
        nc.tensor.matmul(pz[:], ones[:], rs[:], start=True, stop=True)
        z1 = small.tile([1, 1], F32)
        nc.vector.reciprocal(z1[:], pz[:])
        # Euler scalar: alpha/N * (0.5*N/Z) = 0.5*alpha/Z
        sca1 = small.tile([1, 1], F32)
        nc.vector.tensor_scalar_mul(sca1[:], z1[:], 0.5 * ALPHA)
        sca = small.tile([P, 1], F32)
        nc.gpsimd.partition_broadcast(sca[:], sca1[:])
        KSb = big.tile([P, NB, NB, P], BF16, tag="KSb")
        KS = None
        scn = None
        if debug:
            KS = big.tile([P, NB, NB, P], F32, tag="KS")
            scn1 = small.tile([1, 1], F32, tag="scn1")
            nc.vector.tensor_scalar_mul(scn1[:], z1[:], 0.5 * N)
            scn = small.tile([P, 1], F32, tag="scn")
            nc.gpsimd.partition_broadcast(scn[:], scn1[:])
        for a in range(NB):
            for b in range(NB):
                pt = pso.tile([P, P], BF16, tag="pt")
                nc.tensor.transpose(pt[:], ET[:, a, b, :], sIDb[:])
                nc.vector.tensor_add(KSb[:, a, b, :], ET[:, b, a, :], pt[:])
                if debug:
                    nc.vector.scalar_tensor_tensor(
                        out=KS[:, a, b, :], in0=ET[:, b, a, :], scalar=scn[:],
                        in1=pt[:], op0=OP.mult, op1=OP.bypass)
                    nc.vector.scalar_tensor_tensor(
                        out=KS[:, a, b, :], in0=pt[:], scalar=scn[:],
                        in1=KS[:, a, b, :], op0=OP.mult, op1=OP.add)
        if debug:
            nc.sync.dma_start(out=ksym_ext[:],
                              in_=KS[:].rearrange("p a b f -> p a (b f)"))

        # ---- ODE: explicit Euler, fully replicated ----
        traj = big.tile([P, steps, NB], F32, tag="traj")
        th0 = small.tile([P, NB], F32)
        nc.sync.dma_start(out=th0[:], in_=ins["theta0"][:])
        for t in range(steps):
            prev = th0[:] if t == 0 else traj[:, t - 1, :]
            wb = work.tile([P, 16], F32, tag="wb")
            nc.vector.add_range_wrap(out=wb[:, 0:8], in_=prev, shift=0.0,
                                     bound=PI, period=2 * PI)
            nc.vector.add_range_wrap(out=wb[:, 8:16], in_=prev, shift=PI / 2,
                                     bound=PI, period=2 * PI)
            sctb = work.tile([P, NB, 2], BF16, tag="sctb")
            nc.scalar.activation(out=sctb[:].rearrange("p a b -> p b a"),
                                 in_=wb[:], func=AF.Sin)
            po = pso.tile([P, NB, 2], F32, tag="po")
            d1 = work.tile([P, NB], F32, tag="d1")
            d2 = work.tile([P, NB], F32, tag="d2")
            for half in range(2):
                lo, hi = half * 4, half * 4 + 4
                for ib in range(lo, hi):
                    for jb in range(NB):
                        nc.tensor.matmul(po[:, ib, :], KSb[:, jb, ib, :],
                                         sctb[:, jb, :], start=(jb == 0),
                                         stop=(jb == NB - 1))
                nc.vector.tensor_mul(d1[:, lo:hi], sctb[:, lo:hi, 1],
                                     po[:, lo:hi, 0])
                nc.vector.tensor_mul(d2[:, lo:hi], sctb[:, lo:hi, 0],
                                     po[:, lo:hi, 1])
                nc.vector.tensor_sub(d1[:, lo:hi], d1[:, lo:hi], d2[:, lo:hi])
                prev_h = (th0[:, lo:hi] if t == 0 else traj[:, t - 1, lo:hi])
                nc.vector.scalar_tensor_tensor(out=traj[:, t, lo:hi],
                                               in0=d1[:, lo:hi], scalar=sca[:],
                                               in1=prev_h,
                                               op0=OP.mult, op1=OP.add)
        nc.sync.dma_start(out=traj_ext[:], in_=traj[:])

    nc.compile()
    return nc


_CACHED = {}


def _get_program(steps=STEPS, debug=False):
    key = (steps, debug)
    if key not in _CACHED:
        _CACHED[key] = build_program(steps, debug)
    return _CACHED[key]


def make_in_maps(inputs, theta0=None):
    x = np.ascontiguousarray(np.asarray(inputs["x"], dtype=np.float32))
    w1 = np.asarray(inputs["w1"], np.float32)
    if theta0 is None:
        th0 = np.zeros((P, NB), np.float32)
    else:
        th0 = np.ascontiguousarray(
            np.asarray(theta0, np.float32).reshape(NB, P).T)
    base = {
        "xT": np.ascontiguousarray(x.T),
        "x8": np.ascontiguousarray(x.reshape(NB, P, FD)),
        "w1a": np.ascontiguousarray(w1[:FD]),
        "w1b": np.ascontiguousarray(w1[FD:]),
        "w2": np.asarray(inputs["w2"], np.float32),
        "w3l": np.asarray(inputs["w3"], np.float32).reshape(H, 1),
        "b1": np.asarray(inputs["b1"], np.float32).reshape(H, 1),
        "g1": np.asarray(inputs["gamma1"], np.float32).reshape(H, 1),
        "be1": np.asarray(inputs["beta1"], np.float32).reshape(H, 1),
        "g2": np.asarray(inputs["gamma2"], np.float32).reshape(H, 1),
        "be2": np.asarray(inputs["beta2"], np.float32).reshape(H, 1),
        "ident": np.eye(P, dtype=np.float32),
        "theta0": th0,
    }
    maps = []
    for c in range(NCORES):
        m = dict(base)
        m["xbT"] = np.ascontiguousarray(x[c * P:(c + 1) * P].T)
        maps.append(m)
    return maps


def unpack_traj(traj_pb, steps):
    return np.ascontiguousarray(
        traj_pb.transpose(1, 2, 0).reshape(steps, N).astype(np.float32))


def unpack_ksym(ksym_dbg):
    return np.ascontiguousarray(
        ksym_dbg.reshape(P, NB, N).transpose(1, 0, 2).reshape(N, N))


def run(inputs, steps=STEPS, theta0=None, debug=True):
    nc = _get_program(steps, debug)
    res = run_bass_kernel_spmd(nc, make_in_maps(inputs, theta0),
                               list(range(NCORES)))
    return res.results


def kernel(**inputs):
    results = run(inputs, debug=False)
    return unpack_traj(results[0]["traj_pb"], STEPS)


# revision 12
# speedup vs baseline: 1.0700x; 1.0700x over previous
"""KuraNet Trainium2 kernel.

Pipeline (8 NeuronCores, SPMD, core c owns pair-rows i in [128c, 128c+128)):
  - L1 of the pair-MLP is separable: h1[(i,j),f] = x_i.W1A_f + x_j.W1B_f, so it
    reduces to two tiny [1024,16]@[16,128] matmuls (u, v).
  - BN1 batch stats over the N^2 Cartesian pair grid are exact in closed form
    from the 16x16 covariance of x (cross-covariance over the product grid
    vanishes), so no pass over N^2 is needed for BN1.
  - Pass 1 over the core's 131072 pairs (all matmuls bf16): g = Lrelu(av+su_i)
    on ACT (bf16, with accum giving Sum(g) so Sum(h2) = W2^T Sum(g) is free);
    h2 = W2^T g on PE; sumsq(h2) via DVE PSUM->SBUF copy + Pool squaring stt
    (Pool cannot access PSUM).
  - One AllGather of per-core (sum, sumsq) -> exact global BN2 stats.
  - Pass 2: rebuild g, h2 = W2^T g; BN2 affine y = a2*h2+c2 on DVE (PSUM
    read), Lrelu on Pool -> g2 bf16; k = g2^T w3 with g2 blocks stationary so
    k lands partition-major (transposed block layout).
  - One AllGather of k blocks; softmax WITHOUT the max-subtraction pass
    (logits are BN-bounded; exp is computed directly in fp32 and is
    mathematically identical after normalization). The softmax scale
    0.5*N/Z is NOT applied to K at all -- it is folded into the Euler-step
    scalar (alpha*0.5/Z), so the resident coupling matrix is just
    KSb = exp(k) + exp(k)^T in bf16 (built with 64 PE transposes + DVE adds).
  - 150 explicit-Euler Kuramoto steps, replicated per core (no per-step
    collectives): theta' = theta + (0.5a/Z)*(cos th*(KSb@sin th) -
    sin th*(KSb@cos th)); KSb@[s|c] as 64 accumulating [128,128]x[128,2]
    matmuls with KSb blocks stationary (KSb symmetric-transposed blocks give
    the needed transpose for free). sin/cos via one ACT Sin directly into
    bf16 after a DVE range-wrap; combine + Euler update on DVE.
Output traj is produced partition-major [128, 150, 8] and unsharded on host.
"""

import math

import numpy as np

import concourse.bass as bass
import concourse.bacc as bacc
import concourse.tile as tile
import concourse.mybir as mybir
from concourse.bass_utils import run_bass_kernel_spmd

N = 1024
FD = 16
H = 128
P = 128
NB = 8
NCORES = 8
STEPS = 150
ALPHA = 0.1
EPS = 1e-5
SLOPE = 0.01
PI = math.pi
F32 = mybir.dt.float32
BF16 = mybir.dt.bfloat16
AF = mybir.ActivationFunctionType
OP = mybir.AluOpType
AX = mybir.AxisListType


def build_program(steps=STEPS, debug=False):
    nc = bacc.Bacc("TRN2", target_bir_lowering=False, debug=False,
                   num_devices=NCORES)
    ins = {}
    for name, sh in [("xT", [FD, N]), ("x8", [NB, P, FD]), ("xbT", [FD, P]),
                     ("w1a", [FD, H]), ("w1b", [FD, H]), ("w2", [H, H]),
                     ("w3l", [H, 1]), ("b1", [H, 1]), ("g1", [H, 1]),
                     ("be1", [H, 1]), ("g2", [H, 1]), ("be2", [H, 1]),
                     ("ident", [P, P]), ("theta0", [P, NB])]:
        ins[name] = nc.dram_tensor(name, sh, F32, kind="ExternalInput")
    traj_ext = nc.dram_tensor("traj_pb", [P, steps, NB], F32,
                              kind="ExternalOutput")
    ksym_ext = (nc.dram_tensor("ksym_dbg", [P, NB, N], F32,
                               kind="ExternalOutput") if debug else None)

    stats_in = nc.dram_tensor("stats_in", [H, 2], F32)
    stats_sh = nc.dram_tensor("stats_sh", [NCORES, H, 2], F32,
                              addr_space="Shared")
    gstash = nc.dram_tensor("gstash", [P, H, N], mybir.dt.bfloat16)
    k_in = nc.dram_tensor("k_in", [P, N], F32)
    k_sh = nc.dram_tensor("k_sh", [NCORES, P, N], F32, addr_space="Shared")
    rg = [list(range(NCORES))]

    from contextlib import ExitStack
    with tile.TileContext(nc) as tc, ExitStack() as ctx:
        const = ctx.enter_context(tc.tile_pool(name="const", bufs=1))
        big = ctx.enter_context(tc.tile_pool(name="big", bufs=1))
        work = ctx.enter_context(tc.tile_pool(name="work", bufs=3))
        g2p = ctx.enter_context(tc.tile_pool(name="g2p", bufs=2))
        small = ctx.enter_context(tc.tile_pool(name="small", bufs=1))
        ps = ctx.enter_context(tc.tile_pool(name="ps", bufs=2, space="PSUM"))
        pss = ctx.enter_context(tc.tile_pool(name="pss", bufs=1, space="PSUM"))
        psk = ctx.enter_context(tc.tile_pool(name="psk", bufs=1, space="PSUM"))
        pso = ctx.enter_context(tc.tile_pool(name="pso", bufs=1, space="PSUM"))

        def load(name, sh):
            t = const.tile(sh, F32, tag=name)
            nc.sync.dma_start(out=t[:], in_=ins[name][:])
            return t

        sXT = load("xT", [FD, N])
        sXBT = load("xbT", [FD, P])
        sW1A = load("w1a", [FD, H])
        sW1B = load("w1b", [FD, H])
        sW2 = load("w2", [H, H])
        sW3 = load("w3l", [H, 1])
        sB1 = load("b1", [H, 1])
        sG1 = load("g1", [H, 1])
        sBE1 = load("be1", [H, 1])
        sG2 = load("g2", [H, 1])
        sBE2 = load("be2", [H, 1])
        sID = load("ident", [P, P])
        sX8 = const.tile([P, NB, FD], F32, tag="x8")
        nc.sync.dma_start(out=sX8[:], in_=ins["x8"][:].rearrange("b p k -> p b k"))

        # bf16 copies of TensorEngine-facing constants
        sW2b = const.tile([H, H], BF16, tag="w2b")
        nc.vector.tensor_copy(sW2b[:], sW2[:])
        sW3b = const.tile([H, 1], BF16, tag="w3b")
        nc.vector.tensor_copy(sW3b[:], sW3[:])
        sIDb = const.tile([P, P], BF16, tag="identb")
        nc.vector.tensor_copy(sIDb[:], sID[:])

        # ---- BN1 closed-form setup ----
        xb = small.tile([FD, 1], F32)
        nc.vector.tensor_reduce(out=xb[:], in_=sXT[:], axis=AX.X, op=OP.add)
        nc.vector.tensor_scalar_mul(xb[:], xb[:], 1.0 / N)

        mA = small.tile([H, 1], F32)
        mB = small.tile([H, 1], F32)
        for w, m in ((sW1A, mA), (sW1B, mB)):
            pm = pss.tile([H, 1], F32, tag="setup")
            nc.tensor.matmul(pm[:], w[:], xb[:], start=True, stop=True)
            nc.vector.tensor_copy(m[:], pm[:])
        m1 = small.tile([H, 1], F32)   # mu1 + b1
        nc.vector.tensor_add(m1[:], mA[:], mB[:])
        nc.vector.tensor_add(m1[:], m1[:], sB1[:])

        pS = pss.tile([FD, FD], F32, tag="setup")
        for b in range(NB):
            nc.tensor.matmul(pS[:], sX8[:, b, :], sX8[:, b, :],
                             start=(b == 0), stop=(b == NB - 1))
        sS = small.tile([FD, FD], F32)
        nc.vector.tensor_copy(sS[:], pS[:])
        ones = small.tile([P, 1], F32)
        nc.vector.memset(ones[:], 1.0)
        pqs = pss.tile([1, H], F32, tag="setupB")
        for half, w in enumerate((sW1A, sW1B)):
            pSA = pss.tile([FD, H], F32, tag="setup")
            nc.tensor.matmul(pSA[:], sS[:], w[:], start=True, stop=True)
            qa = small.tile([FD, H], F32, tag=f"qa{half}")
            nc.vector.tensor_mul(qa[:], pSA[:], w[:])
            nc.tensor.matmul(pqs[:], ones[0:FD, :], qa[:],
                             start=(half == 0), stop=(half == 1))
        qsum = small.tile([1, H], F32)
        nc.vector.tensor_copy(qsum[:], pqs[:])
        pq = pss.tile([H, 1], F32, tag="setup")
        nc.tensor.transpose(pq[:], qsum[:], sID[0:1, 0:1])
        t1 = small.tile([H, 1], F32, tag="t1")
        nc.vector.tensor_mul(t1[:], mA[:], mA[:])
        var1 = small.tile([H, 1], F32)
        nc.vector.scalar_tensor_tensor(out=var1[:], in0=pq[:], scalar=1.0 / N,
                                       in1=t1[:], op0=OP.mult, op1=OP.subtract)
        nc.vector.tensor_mul(t1[:], mB[:], mB[:])
        nc.vector.tensor_sub(var1[:], var1[:], t1[:])
        eps_t = small.tile([H, 1], F32)
        nc.vector.memset(eps_t[:], EPS)
        sd = small.tile([H, 1], F32)
        nc.scalar.activation(out=sd[:], in_=var1[:], func=AF.Sqrt, bias=eps_t[:])
        a1 = small.tile([H, 1], F32)
        nc.vector.reciprocal(a1[:], sd[:])
        nc.vector.tensor_mul(a1[:], a1[:], sG1[:])
        c1 = small.tile([H, 1], F32)
        nc.vector.tensor_mul(c1[:], a1[:], m1[:])
        nc.vector.tensor_sub(c1[:], sBE1[:], c1[:])

        pu = pss.tile([H, P], F32, tag="setup")
        nc.tensor.matmul(pu[:], sW1A[:], sXBT[:], start=True, stop=True)
        su = const.tile([H, P], F32, tag="su")
        nc.scalar.activation(out=su[:], in_=pu[:], func=AF.Identity,
                             bias=c1[:], scale=a1[:])
        av = const.tile([H, N], F32, tag="av")
        for h in range(2):
            pv = pss.tile([H, 512], F32, tag="setup")
            nc.tensor.matmul(pv[:], sW1B[:], sXT[:, h * 512:(h + 1) * 512],
                             start=True, stop=True)
            nc.scalar.activation(out=av[:, h * 512:(h + 1) * 512], in_=pv[:],
                                 func=AF.Identity, scale=a1[:])

        # ---- pass 1: BN2 moments via DVE bn_stats (the only engine that can
        # both read PSUM and form mean/var in one pass) ----
        stats = big.tile([P, P, 2, 6], F32, tag="stats")
        for i in range(P):
            g = work.tile([H, N], BF16, tag="g")
            nc.scalar.activation(out=g[:], in_=av[:], func=AF.Lrelu,
                                 bias=su[:, i:i + 1], alpha=SLOPE)
            nc.sync.dma_start(out=gstash[i], in_=g[:])
            for h in range(2):
                ph2 = ps.tile([H, 512], F32, tag="ph2")
                nc.tensor.matmul(ph2[:], sW2b[:], g[:, h * 512:(h + 1) * 512],
                                 start=True, stop=True)
                nc.vector.bn_stats(out=stats[:, i, h, :], in_=ph2[:])
        mv = small.tile([H, 2], F32)
        nc.vector.bn_aggr(out=mv[:], in_=stats[:].rearrange("p i h s -> p (i h) s"))
        CNT = float(P * N)
        ex = small.tile([H, 2], F32)
        nc.vector.tensor_scalar_mul(ex[:, 0:1], mv[:, 0:1], CNT)
        tq = small.tile([H, 1], F32, tag="tq")
        nc.vector.tensor_mul(tq[:], mv[:, 0:1], mv[:, 0:1])
        nc.vector.tensor_add(tq[:], tq[:], mv[:, 1:2])
        nc.vector.tensor_scalar_mul(ex[:, 1:2], tq[:], CNT)
        nc.sync.dma_start(out=stats_in[:], in_=ex[:])
        nc.gpsimd.collective_compute("AllGather", OP.bypass, replica_groups=rg,
                                     ins=[stats_in[:]], outs=[stats_sh[:]])
        sg = small.tile([H, NCORES, 2], F32)
        nc.sync.dma_start(out=sg[:], in_=stats_sh[:].rearrange("r p s -> p r s"))
        tot = small.tile([H, 2], F32)
        nc.vector.tensor_reduce(out=tot[:, 0:1], in_=sg[:, :, 0], axis=AX.X,
                                op=OP.add)
        nc.vector.tensor_reduce(out=tot[:, 1:2], in_=sg[:, :, 1], axis=AX.X,
                                op=OP.add)
        TOT = float(NCORES * P * N)
        mean2 = small.tile([H, 1], F32)
        nc.vector.tensor_scalar_mul(mean2[:], tot[:, 0:1], 1.0 / TOT)
        var2 = small.tile([H, 1], F32)
        nc.vector.tensor_scalar_mul(var2[:], tot[:, 1:2], 1.0 / TOT)
        tm = small.tile([H, 1], F32, tag="tm")
        nc.vector.tensor_mul(tm[:], mean2[:], mean2[:])
        nc.vector.tensor_sub(var2[:], var2[:], tm[:])
        sd2 = small.tile([H, 1], F32)
        nc.scalar.activation(out=sd2[:], in_=var2[:], func=AF.Sqrt, bias=eps_t[:])
        a2 = small.tile([H, 1], F32)
        nc.vector.reciprocal(a2[:], sd2[:])
        nc.vector.tensor_mul(a2[:], a2[:], sG2[:])
        c2 = small.tile([H, 1], F32)
        nc.vector.tensor_mul(c2[:], a2[:], mean2[:])
        nc.vector.tensor_sub(c2[:], sBE2[:], c2[:])

        # ---- pass 2: k logits (transposed block layout) ----
        pkb0 = psk.tile([P, NB, 64], F32, tag="pk0")
        pkb1 = psk.tile([P, NB, 64], F32, tag="pk1")
        pkb = [pkb0, pkb1]

        def kmms(i, g2t):
            bank, slot = divmod(i, 64)
            for jb in range(NB):
                nc.tensor.matmul(pkb[bank][:, jb, slot:slot + 1],
                                 g2t[:, jb * P:(jb + 1) * P], sW3b[:],
                                 start=True, stop=True)

        # g comes back from DRAM (stashed in pass 1); k-matmuls of iteration
        # i-1 are emitted after the h2 matmuls of iteration i so the PE never
        # stalls waiting for the elementwise BN2 path.
        def fetch_g(i):
            gi = work.tile([H, N], BF16, tag="gi")
            nc.sync.dma_start(out=gi[:], in_=gstash[i])
            return gi

        g_cur = fetch_g(0)
        g2_prev = None
        for i in range(P):
            ph2a = ps.tile([H, 512], F32, tag="ph2")
            nc.tensor.matmul(ph2a[:], sW2b[:], g_cur[:, 0:512],
                             start=True, stop=True)
            ph2b = ps.tile([H, 512], F32, tag="ph2")
            nc.tensor.matmul(ph2b[:], sW2b[:], g_cur[:, 512:1024],
                             start=True, stop=True)
            if i + 1 < P:
                g_cur = fetch_g(i + 1)
            if g2_prev is not None:
                kmms(i - 1, g2_prev)
            g2t = g2p.tile([H, N], BF16, tag="g2")
            # ACT: fused BN2 affine + Lrelu on half A and cols [512:640];
            # DVE 2-op path covers [640:1024].
            nc.scalar.activation(out=g2t[:, 0:512], in_=ph2a[:],
                                 func=AF.Lrelu, bias=c2[:], scale=a2[:],
                                 alpha=SLOPE)
            nc.scalar.activation(out=g2t[:, 512:640], in_=ph2b[:, 0:128],
                                 func=AF.Lrelu, bias=c2[:], scale=a2[:],
                                 alpha=SLOPE)
            yt = work.tile([H, 384], F32, tag="yt")
            nc.vector.tensor_scalar(out=yt[:], in0=ph2b[:, 128:512],
                                    scalar1=a2[:], scalar2=c2[:],
                                    op0=OP.mult, op1=OP.add)
            nc.vector.scalar_tensor_tensor(out=g2t[:, 640:1024],
                                           in0=yt[:], scalar=SLOPE, in1=yt[:],
                                           op0=OP.mult, op1=OP.max)
            g2_prev = g2t
        kmms(P - 1, g2_prev)
        KT = big.tile([P, NB, P], F32, tag="KT")
        for bank in range(2):
            nc.vector.tensor_copy(KT[:, :, bank * 64:(bank + 1) * 64],
                                  pkb[bank][:])
        nc.sync.dma_start(out=k_in[:], in_=KT[:].rearrange("p j f -> p (j f)"))
        nc.gpsimd.collective_compute("AllGather", OP.bypass, replica_groups=rg,
                                     ins=[k_in[:]], outs=[k_sh[:]])
        # kallT[p, r, s, f] = k(128r+f, 128s+p)
        kallT = big.tile([P, NB, NB, P], F32, tag="kallT")
        nc.sync.dma_start(out=kallT[:],
                          in_=k_sh[:].rearrange("r p (s f) -> p r s f", s=NB))

        # ---- exp (no max pass; normalization folded into the step scalar),
        #      symmetrize: KSb_blk(a,b) = exp(k)_blk(b,a) + T(exp(k)_blk(a,b))
        ET = big.tile([P, NB, NB, P], BF16, tag="ET")
        es = small.tile([P, NB], F32)
        for r in range(NB):
            nc.scalar.activation(out=ET[:, r, :, :], in_=kallT[:, r, :, :],
                                 func=AF.Exp, accum_out=es[:, r:r + 1])
        rs = small.tile([P, 1], F32)
        nc.vector.tensor_reduce(out=rs[:], in_=es[:], axis=AX.X, op=OP.add)
        pz = pss.tile([1, 1], F32, tag="setup")
        nc.tensor.matmul(pz[:], ones[:], rs[:], start=True, stop=True)
        z1 = small.tile([1, 1], F32)
        nc.vector.reciprocal(z1[:], pz[:])
        # Euler scalar: alpha/N * (0.5*N/Z) = 0.5*alpha/Z
        sca1 = small.tile([1, 1], F32)
        nc.vector.tensor_scalar_mul(sca1[:], z1[:], 0.5 * ALPHA)
        sca = small.tile([P, 1], F32)
        nc.gpsimd.partition_broadcast(sca[:], sca1[:])
        KSb = big.tile([P, NB, NB, P], BF16, tag="KSb")
        KS = None
        scn = None
        if debug:
            KS = big.tile([P, NB, NB, P], F32, tag="KS")
            scn1 = small.tile([1, 1], F32, tag="scn1")
            nc.vector.tensor_scalar_mul(scn1[:], z1[:], 0.5 * N)
            scn = small.tile([P, 1], F32, tag="scn")
            nc.gpsimd.partition_broadcast(scn[:], scn1[:])
        for a in range(NB):
            for b in range(NB):
                pt = pso.tile([P, P], BF16, tag="pt")
                nc.tensor.transpose(pt[:], ET[:, a, b, :], sIDb[:])
                nc.vector.tensor_add(KSb[:, a, b, :], ET[:, b, a, :], pt[:])
                if debug:
                    nc.vector.scalar_tensor_tensor(
                        out=KS[:, a, b, :], in0=ET[:, b, a, :], scalar=scn[:],
                        in1=pt[:], op0=OP.mult, op1=OP.bypass)
                    nc.vector.scalar_tensor_tensor(
                        out=KS[:, a, b, :], in0=pt[:], scalar=scn[:],
                        in1=KS[:, a, b, :], op0=OP.mult, op1=OP.add)
        if debug:
            nc.sync.dma_start(out=ksym_ext[:],
                              in_=KS[:].rearrange("p a b f -> p a (b f)"))

        # ---- ODE: explicit Euler, fully replicated ----
        traj = big.tile([P, steps, NB], F32, tag="traj")
        th0 = small.tile([P, NB], F32)
        nc.sync.dma_start(out=th0[:], in_=ins["theta0"][:])
        for t in range(steps):
            prev = th0[:] if t == 0 else traj[:, t - 1, :]
            wb = work.tile([P, 16], F32, tag="wb")
            nc.vector.add_range_wrap(out=wb[:, 0:8], in_=prev, shift=0.0,
                                     bound=PI, period=2 * PI)
            nc.vector.add_range_wrap(out=wb[:, 8:16], in_=prev, shift=PI / 2,
                                     bound=PI, period=2 * PI)
            sctb = work.tile([P, NB, 2], BF16, tag="sctb")
            nc.scalar.activation(out=sctb[:].rearrange("p a b -> p b a"),
                                 in_=wb[:], func=AF.Sin)
            po = pso.tile([P, NB, 2], F32, tag="po")
            for ib in range(NB):
                for jb in range(NB):
                    nc.tensor.matmul(po[:, ib, :], KSb[:, jb, ib, :],
                                     sctb[:, jb, :], start=(jb == 0),
                                     stop=(jb == NB - 1))
            d1 = work.tile([P, NB], F32, tag="d1")
            nc.vector.tensor_mul(d1[:], sctb[:, :, 1], po[:, :, 0])
            d2 = work.tile([P, NB], F32, tag="d2")
            nc.vector.tensor_mul(d2[:], sctb[:, :, 0], po[:, :, 1])
            nc.vector.tensor_sub(d1[:], d1[:], d2[:])
            nc.vector.scalar_tensor_tensor(out=traj[:, t, :], in0=d1[:],
                                           scalar=sca[:], in1=prev,
                                           op0=OP.mult, op1=OP.add)
        nc.sync.dma_start(out=traj_ext[:], in_=traj[:])

    nc.compile()
    return nc


_CACHED = {}


def _get_program(steps=STEPS, debug=False):
    key = (steps, debug)
    if key not in _CACHED:
        _CACHED[key] = build_program(steps, debug)
    return _CACHED[key]


def make_in_maps(inputs, theta0=None):
    x = np.ascontiguousarray(np.asarray(inputs["x"], dtype=np.float32))
    w1 = np.asarray(inputs["w1"], np.float32)
    if theta0 is None:
        th0 = np.zeros((P, NB), np.float32)
    else:
        th0 = np.ascontiguousarray(
            np.asarray(theta0, np.float32).reshape(NB, P).T)
    base = {
        "xT": np.ascontiguousarray(x.T),
        "x8": np.ascontiguousarray(x.reshape(NB, P, FD)),
        "w1a": np.ascontiguousarray(w1[:FD]),
        "w1b": np.ascontiguousarray(w1[FD:]),
        "w2": np.asarray(inputs["w2"], np.float32),
        "w3l": np.asarray(inputs["w3"], np.float32).reshape(H, 1),
        "b1": np.asarray(inputs["b1"], np.float32).reshape(H, 1),
        "g1": np.asarray(inputs["gamma1"], np.float32).reshape(H, 1),
        "be1": np.asarray(inputs["beta1"], np.float32).reshape(H, 1),
        "g2": np.asarray(inputs["gamma2"], np.float32).reshape(H, 1),
        "be2": np.asarray(inputs["beta2"], np.float32).reshape(H, 1),
        "ident": np.eye(P, dtype=np.float32),
        "theta0": th0,
    }
    maps = []
    for c in range(NCORES):
        m = dict(base)
        m["xbT"] = np.ascontiguousarray(x[c * P:(c + 1) * P].T)
        maps.append(m)
    return maps


def unpack_traj(traj_pb, steps):
    return np.ascontiguousarray(
        traj_pb.transpose(1, 2, 0).reshape(steps, N).astype(np.float32))


def unpack_ksym(ksym_dbg):
    return np.ascontiguousarray(
        ksym_dbg.reshape(P, NB, N).transpose(1, 0, 2).reshape(N, N))


def run(inputs, steps=STEPS, theta0=None, debug=True):
    nc = _get_program(steps, debug)
    res = run_bass_kernel_spmd(nc, make_in_maps(inputs, theta0),
                               list(range(NCORES)))
    return res.results


def kernel(**inputs):
    results = run(inputs, debug=False)
    return unpack_traj(results[0]["traj_pb"], STEPS)
